# revision 2
# baseline (speedup 1.0000x reference)
"""Distributed euclidean-distance loss kernel for Trainium2 (8 NeuronCores).

loss = sum_i sqrt(sum_c (preds[i,c] - targets[i,c])^2) / (N + 1)

preds/targets: [16777216, 2] f32. Data-parallel over the batch axis: each
of the 8 cores reduces its 1/8 slice to per-partition partial sums; the
host sums the partials and divides by N+1.

DMA-engine load balancing: HWDGE splits a transfer's descriptors into
ceil(n/16) chunks handed to SDMA engines 0,1,2,... in order, so a
128-row transfer loads all 16 engines evenly while a 120-row transfer
loads only engines 0-14. Engine 15 (the dynamic-queue service engine) is
~20-25% slower than the rest, so the input is split into two streams:

  stream A: [128 partitions x WA]  -> all 16 engines (8 rows each)
  stream B: [120 partitions x WB]  -> engines 0-14 only

with widths chosen so engine 15 (A only) and engines 0-14 (A+B) finish
together. Each stream has its own tiles, buffers, and compute chain
(sqdiff -> pair-add -> sqrt-accumulate); zero padding is never needed
because both streams are rectangular.
"""

from contextlib import ExitStack

import numpy as np

import concourse.bass as bass
import concourse.bacc as bacc
import concourse.mybir as mybir
from concourse import dve_ops
from concourse.bass_utils import run_bass_kernel_spmd
from concourse.dve_spec import Spec, Src0, Src1, _has_src1, lower, sq
from concourse.dve_uop import DveOpSpec

N_CORES = 8
N_POINTS = 16777216
PTS_PER_CORE = N_POINTS // N_CORES       # 2_097_152
E = PTS_PER_CORE * 2                     # 4_194_304 f32 per tensor per core
P = 128                                  # SBUF partitions
PB = 120                                 # partitions used by stream B

# Per-partition per-tensor element counts. Must satisfy:
#   P * MA + PB * MB == E
# MA/(MA+MB) sets engine-15's relative load (~0.78 of the others).
NT_A = 16
WA = 1628                                # A-tile width (per tensor)
MA = NT_A * WA                           # 26048
NT_B = 4
WB = 1792                                # B-tile width (per tensor)
MB = NT_B * WB                           # 7168
assert P * MA + PB * MB == E

_cache = {}


def _register_sqdiff():
    """Custom DVE op out = (in0 - in1)^2: one Vector instruction for the
    subtract+square."""
    name = "SQDIFF_DIST_ANT"
    for op in dve_ops.OPS:
        if op.name == name:
            return op
    spec = Spec(
        body=sq(Src0 - Src1),
        reference=lambda in0, in1, s0, s1, imm2: (
            (in0.astype(np.float32) - in1) ** 2
        ).astype(np.float32),
    )
    row = max(dve_ops._SUB_OPCODE_FOR_NAME.values()) + 1
    assert row < 0x20
    shas = {}
    for ver in ("v3", "v4"):
        uops = lower(spec, ver=ver)
        shas[ver] = DveOpSpec(
            name=name, opcode=row, uops=uops, rd1_en=_has_src1(spec)
        ).sha(ver)
    op = dve_ops.DveOp(name, spec, subdim=False, uops_sha=shas)
    dve_ops.OPS.append(op)
    dve_ops._SUB_OPCODE_FOR_NAME[name] = row
    dve_ops.CUSTOM_DVE_SPECS[name] = spec
    return op


_SQDIFF = _register_sqdiff()


def _a_tiles(taper=True):
    """A-stream tile list as (elem_offset, width). With taper the last
    full tile is split in half so the end-of-stream compute chain
    shrinks."""
    out = [(i * WA, WA) for i in range(NT_A)]
    if taper:
        off, w = out.pop()
        out += [(off, w // 2), (off + w // 2, w // 2)]
    return out


def _schedule(taper=True):
    """Global issue/processing order: interleave one B-tile after every
    4 A-tiles, with the B stream finishing before the final (tapered)
    A-tiles so the tail chain is short."""
    at = _a_tiles(taper)
    order = []
    ai = 0
    for r in range(NT_B):
        take = 4 if r < NT_B - 1 else max(0, len(at) - 2 - ai)
        for _ in range(take):
            order.append(("A", ai, at[ai]))
            ai += 1
        order.append(("B", r, (r * WB, WB)))
    while ai < len(at):
        order.append(("A", ai, at[ai]))
        ai += 1
    return order, len(at)


def _build(nba=8, nbb=3, taper=True, out_wait=False):
    """Raw bacc build: Sync issues input DMAs (single HWDGE ring, FIFO),
    Vector runs sqdiff + pair-add, Scalar runs sqrt with accumulate."""
    order, nta = _schedule(taper)
    fp32 = mybir.dt.float32
    nc = bacc.Bacc(
        "TRN2", target_bir_lowering=False, debug=False, num_devices=N_CORES,
        enable_partition_id=False,
    )
    xa_in = nc.declare_dram_parameter("xa", [P, 2 * MA], fp32, isOutput=False)
    xb_in = nc.declare_dram_parameter("xb", [PB, 2 * MB], fp32, isOutput=False)
    oa = nc.declare_dram_parameter("oa", [P, nta], fp32, isOutput=True)
    ob = nc.declare_dram_parameter("ob", [PB, NT_B], fp32, isOutput=True)

    with ExitStack() as ctx:
        xta = [
            ctx.enter_context(nc.sbuf_tensor(f"xta{j}", [P, 2 * WA], fp32))
            for j in range(nba)
        ]
        xtb = [
            ctx.enter_context(nc.sbuf_tensor(f"xtb{j}", [PB, 2 * WB], fp32))
            for j in range(nbb)
        ]
        sqa = ctx.enter_context(nc.sbuf_tensor("sqa", [P, WA], fp32))
        sqb = ctx.enter_context(nc.sbuf_tensor("sqb", [PB, WB], fp32))
        psa = [
            ctx.enter_context(nc.sbuf_tensor(f"psa{j}", [P, WA // 2], fp32))
            for j in range(2)
        ]
        psb = [
            ctx.enter_context(nc.sbuf_tensor(f"psb{j}", [PB, WB // 2], fp32))
            for j in range(2)
        ]
        acca = ctx.enter_context(nc.sbuf_tensor("acca", [P, nta], fp32))
        accb = ctx.enter_context(nc.sbuf_tensor("accb", [PB, NT_B], fp32))
        dma_a = [
            ctx.enter_context(nc.semaphore(f"dma_a{j}")) for j in range(nba)
        ]
        dma_b = [
            ctx.enter_context(nc.semaphore(f"dma_b{j}")) for j in range(nbb)
        ]
        out_sem = ctx.enter_context(nc.semaphore("out_sem"))
        vec_a = ctx.enter_context(nc.semaphore("vec_a"))
        vec_b = ctx.enter_context(nc.semaphore("vec_b"))
        act_a = ctx.enter_context(nc.semaphore("act_a"))
        act_b = ctx.enter_context(nc.semaphore("act_b"))

        with nc.Block(no_gpsimd_drain=True) as block:

            @block.sync
            def _(sync):
                for kind, i, (off, w) in order:
                    if kind == "A":
                        if i >= nba:
                            sync.wait_ge(vec_a, 2 * (i - nba) + 1)
                        sync.dma_start(
                            xta[i % nba][:, : 2 * w],
                            xa_in[:, 2 * off : 2 * (off + w)],
                        ).then_inc(dma_a[i % nba], 16)
                    else:
                        if i >= nbb:
                            sync.wait_ge(vec_b, 2 * (i - nbb) + 1)
                        sync.dma_start(
                            xtb[i % nbb][:, : 2 * w],
                            xb_in[:, 2 * off : 2 * (off + w)],
                        ).then_inc(dma_b[i % nbb], 16)
                sync.wait_ge(act_a, nta)
                sync.wait_ge(act_b, NT_B)
                sync.dma_start(oa[:], acca[:]).then_inc(out_sem, 16)
                sync.dma_start(ob[:], accb[:]).then_inc(out_sem, 16)
                if out_wait:
                    sync.wait_ge(out_sem, 32)

            @block.vector
            def _(vector):
                for kind, i, (off, w) in order:
                    if kind == "A":
                        xt, sqt, ps, dma, vec, act, nb = (
                            xta, sqa, psa, dma_a, vec_a, act_a, nba
                        )
                    else:
                        xt, sqt, ps, dma, vec, act, nb = (
                            xtb, sqb, psb, dma_b, vec_b, act_b, nbb
                        )
                    vector.wait_ge(dma[i % nb], 16 * (i // nb + 1))
                    nc.vector._custom_dve(
                        _SQDIFF,
                        out=sqt[:, :w],
                        in0=xt[i % nb][:, :w],
                        in1=xt[i % nb][:, w : 2 * w],
                    ).then_inc(vec, 1)
                    vector.wait_ge(vec, 2 * i + 1)
                    if i >= 2:
                        vector.wait_ge(act, i - 1)
                    nc.vector.tensor_add(
                        ps[i % 2][:, : w // 2],
                        sqt[:, 0:w:2],
                        sqt[:, 1:w:2],
                    ).then_inc(vec, 1)

            @block.scalar
            def _(scalar):
                for kind, i, (off, w) in order:
                    if kind == "A":
                        ps, vec, act, acc = psa, vec_a, act_a, acca
                    else:
                        ps, vec, act, acc = psb, vec_b, act_b, accb
                    scalar.wait_ge(vec, 2 * (i + 1))
                    nc.scalar.activation(
                        ps[i % 2][:, : w // 2],
                        ps[i % 2][:, : w // 2],
                        mybir.ActivationFunctionType.Sqrt,
                        accum_out=acc[:, i : i + 1],
                    ).then_inc(act, 1)

    nc.compile()
    return nc


def _pack(preds, targets, taper=True):
    """[N,2]x2 f32 -> per-core (xa [128, 2*MA], xb [120, 2*MB]).

    Per core each tensor's flat slice is split: partitions 0-119 get
    MA+MB contiguous elements, partitions 120-127 get MA. The first MA
    of each row feed stream A, the rest stream B. Within a stream, tile
    t's chunk is [preds_w | targets_w]."""
    MF = MA + MB
    p4 = np.ascontiguousarray(preds, dtype=np.float32).reshape(N_CORES, E)
    t4 = np.ascontiguousarray(targets, dtype=np.float32).reshape(N_CORES, E)

    def split(arr):
        fast = arr[:, : PB * MF].reshape(N_CORES, PB, MF)
        slow = arr[:, PB * MF :].reshape(N_CORES, P - PB, MA)
        a = np.concatenate([fast[:, :, :MA], slow], axis=1)   # [C,128,MA]
        b = fast[:, :, MA:]                                    # [C,120,MB]
        return a, b

    pa, pb = split(p4)
    ta, tb = split(t4)

    xa = np.empty((N_CORES, P, 2 * MA), dtype=np.float32)
    for off, w in _a_tiles(taper):
        xa[:, :, 2 * off : 2 * off + w] = pa[:, :, off : off + w]
        xa[:, :, 2 * off + w : 2 * (off + w)] = ta[:, :, off : off + w]
    xb = np.empty((N_CORES, PB, 2 * MB), dtype=np.float32)
    for r in range(NT_B):
        off = r * WB
        xb[:, :, 2 * off : 2 * off + WB] = pb[:, :, off : off + WB]
        xb[:, :, 2 * off + WB : 2 * (off + WB)] = tb[:, :, off : off + WB]
    return xa, xb


def _run(preds, targets, nba=8, nbb=3, taper=True, out_wait=False,
         **run_kwargs):
    """Shard, run on hardware, return (partials [n_cores, ncols], result)."""
    key = (nba, nbb, taper, out_wait)
    if key not in _cache:
        _cache[key] = _build(nba=nba, nbb=nbb, taper=taper, out_wait=out_wait)
    nc = _cache[key]
    xa, xb = _pack(preds, targets, taper=taper)
    in_maps = [{"xa": xa[c], "xb": xb[c]} for c in range(N_CORES)]
    r = run_bass_kernel_spmd(
        nc, in_maps, core_ids=list(range(N_CORES)), **run_kwargs
    )
    partials = np.concatenate(
        [
            np.stack([r.results[c]["oa"].ravel() for c in range(N_CORES)]),
            np.stack([r.results[c]["ob"].ravel() for c in range(N_CORES)]),
        ],
        axis=1,
    )
    return partials, r


def kernel(preds, targets):
    import os

    # Force tracing off: the NTFF profile hook isn't importable in a bare
    # container and BASS_TRACE=1 in the environment would crash the run.
    prev = os.environ.get("BASS_NEVER_TRACE")
    os.environ["BASS_NEVER_TRACE"] = "1"
    try:
        partials, _ = _run(preds, targets)
    finally:
        if prev is None:
            os.environ.pop("BASS_NEVER_TRACE", None)
        else:
            os.environ["BASS_NEVER_TRACE"] = prev
    n = preds.shape[0]
    loss = partials.astype(np.float64).sum() / np.float64(n + 1)
    return np.float32(loss)


# revision 3
# speedup vs baseline: 1.0376x; 1.0376x over previous
"""Distributed euclidean-distance loss kernel for Trainium2 (8 NeuronCores).

loss = sum_i sqrt(sum_c (preds[i,c] - targets[i,c])^2) / (N + 1)

preds/targets: [16777216, 2] f32. Data-parallel over the batch axis: each
of the 8 cores reduces its 1/8 slice to a per-partition partial sum; the
host sums the 8 cores' partials and divides by N+1.

The kernel is chip-HBM-bound (~2.6 TB/s effective across 8 streaming
cores). Only full-128-partition HWDGE transfers reach full DMA rate
(partial-partition transfers run at half rate due to SBUF-port
collisions), so the stream is uniform [128, 2f] tiles; per-core host
packing interleaves [preds_tile | targets_tile] per partition row so
each tile is one large contiguous-descriptor DMA. Deep buffering (8
slots) rides through multi-microsecond HBM arbitration stalls, and the
final tile is split in half (8KB descriptors, never smaller) to shorten
the end-of-stream compute chain.
"""

from contextlib import ExitStack

import numpy as np

import concourse.bass as bass
import concourse.bacc as bacc
import concourse.mybir as mybir
from concourse import dve_ops
from concourse.bass_utils import run_bass_kernel_spmd
from concourse.dve_spec import Spec, Src0, Src1, _has_src1, lower, sq
from concourse.dve_uop import DveOpSpec

N_CORES = 8
N_POINTS = 16777216
PTS_PER_CORE = N_POINTS // N_CORES          # 2_097_152
P = 128                                      # SBUF partitions
M = PTS_PER_CORE * 2 // P                    # 32768 floats per partition
F = 2048                                     # tile free size per tensor

_cache = {}


def _register_sqdiff():
    """Custom DVE op out = (in0 - in1)^2 so the subtract+square is one
    Vector instruction."""
    name = "SQDIFF_DIST_ANT"
    for op in dve_ops.OPS:
        if op.name == name:
            return op
    spec = Spec(
        body=sq(Src0 - Src1),
        reference=lambda in0, in1, s0, s1, imm2: (
            (in0.astype(np.float32) - in1) ** 2
        ).astype(np.float32),
    )
    row = max(dve_ops._SUB_OPCODE_FOR_NAME.values()) + 1
    assert row < 0x20
    shas = {}
    for ver in ("v3", "v4"):
        uops = lower(spec, ver=ver)
        shas[ver] = DveOpSpec(
            name=name, opcode=row, uops=uops, rd1_en=_has_src1(spec)
        ).sha(ver)
    op = dve_ops.DveOp(name, spec, subdim=False, uops_sha=shas)
    dve_ops.OPS.append(op)
    dve_ops._SUB_OPCODE_FOR_NAME[name] = row
    dve_ops.CUSTOM_DVE_SPECS[name] = spec
    return op


_SQDIFF = _register_sqdiff()


def _tiles(m, f, taper):
    """Tile list as (elem_offset, free_size) per tensor. With taper the
    last tile is split in half (descriptors stay >= 8KB)."""
    ntiles = m // f
    out = [(i * f, f) for i in range(ntiles)]
    if taper and ntiles >= 2 and f % 2 == 0:
        off, sz = out.pop()
        out += [(off, sz // 2), (off + sz // 2, sz // 2)]
    return out


def _build(m=M, f=F, nb=8, pb=2, taper=True, out_split=True, out_wait=False):
    """Raw bacc build: Sync issues input DMAs (HWDGE ring, FIFO
    completion order), Vector runs sqdiff + pair-add, Scalar runs sqrt
    with accumulate. Output is the per-tile accumulator columns; the
    host does the final cross-tile/cross-partition sum."""
    tiles = _tiles(m, f, taper)
    T = len(tiles)
    fp32 = mybir.dt.float32
    nc = bacc.Bacc(
        "TRN2", target_bir_lowering=False, debug=False, num_devices=N_CORES,
        enable_partition_id=False,
    )
    x_in = nc.declare_dram_parameter("x", [P, 2 * m], fp32, isOutput=False)
    out = nc.declare_dram_parameter("o", [P, T], fp32, isOutput=True)
    with ExitStack() as ctx:
        xt = [
            ctx.enter_context(nc.sbuf_tensor(f"xt{j}", [P, 2 * f], fp32))
            for j in range(nb)
        ]
        sqt = ctx.enter_context(nc.sbuf_tensor("sq", [P, f], fp32))
        ps = [
            ctx.enter_context(nc.sbuf_tensor(f"ps{j}", [P, f // 2], fp32))
            for j in range(pb)
        ]
        acc = ctx.enter_context(nc.sbuf_tensor("acc", [P, T], fp32))
        dma_sems = [
            ctx.enter_context(nc.semaphore(f"dma_sem{j}")) for j in range(nb)
        ]
        out_sem = ctx.enter_context(nc.semaphore("out_sem"))
        vec_sem = ctx.enter_context(nc.semaphore("vec_sem"))
        act_sem = ctx.enter_context(nc.semaphore("act_sem"))

        with nc.Block(no_gpsimd_drain=True) as block:

            @block.sync
            def _(sync):
                for i, (off, sz) in enumerate(tiles):
                    if i >= nb:
                        # xt slot free once sqdiff of tile i-nb has read it
                        sync.wait_ge(vec_sem, 2 * (i - nb) + 1)
                    sync.dma_start(
                        xt[i % nb][:, : 2 * sz],
                        x_in[:, 2 * off : 2 * (off + sz)],
                    ).then_inc(dma_sems[i % nb], 16)
                if out_split:
                    # overlap the bulk of the output transfer with the
                    # last tiles' compute; only the final 2 columns ride
                    # the critical path
                    sync.wait_ge(act_sem, T - 2)
                    sync.dma_start(
                        out[:, : T - 2], acc[:, : T - 2]
                    ).then_inc(out_sem, 16)
                    sync.wait_ge(act_sem, T)
                    sync.dma_start(
                        out[:, T - 2 :], acc[:, T - 2 :]
                    ).then_inc(out_sem, 16)
                    if out_wait:
                        sync.wait_ge(out_sem, 32)
                else:
                    sync.wait_ge(act_sem, T)
                    sync.dma_start(out[:], acc[:]).then_inc(out_sem, 16)
                    if out_wait:
                        sync.wait_ge(out_sem, 16)

            @block.vector
            def _(vector):
                for i, (off, sz) in enumerate(tiles):
                    vector.wait_ge(dma_sems[i % nb], 16 * (i // nb + 1))
                    nc.vector._custom_dve(
                        _SQDIFF,
                        out=sqt[:, :sz],
                        in0=xt[i % nb][:, :sz],
                        in1=xt[i % nb][:, sz : 2 * sz],
                    ).then_inc(vec_sem, 1)
                    # same-engine RAW on sq; HW drains this anyway, but
                    # the race detector wants the sem
                    vector.wait_ge(vec_sem, 2 * i + 1)
                    if i >= pb:
                        # ps slot free once sqrt of tile i-pb consumed it
                        vector.wait_ge(act_sem, i - pb + 1)
                    nc.vector.tensor_add(
                        ps[i % pb][:, : sz // 2],
                        sqt[:, 0:sz:2],
                        sqt[:, 1:sz:2],
                    ).then_inc(vec_sem, 1)

            @block.scalar
            def _(scalar):
                for i, (off, sz) in enumerate(tiles):
                    scalar.wait_ge(vec_sem, 2 * (i + 1))
                    nc.scalar.activation(
                        ps[i % pb][:, : sz // 2],
                        ps[i % pb][:, : sz // 2],
                        mybir.ActivationFunctionType.Sqrt,
                        accum_out=acc[:, i : i + 1],
                    ).then_inc(act_sem, 1)

    nc.compile()
    return nc


def _pack(preds, targets, m, f, n_cores, taper=True):
    """[N,2]x2 f32 -> per-core interleaved [n_cores, P, 2m]: for each
    tile (off, sz), the preds chunk then the targets chunk, matching the
    kernel's slicing."""
    p3 = np.ascontiguousarray(preds, dtype=np.float32).reshape(n_cores, P, m)
    t3 = np.ascontiguousarray(targets, dtype=np.float32).reshape(n_cores, P, m)
    x = np.empty((n_cores, P, 2 * m), dtype=np.float32)
    for off, sz in _tiles(m, f, taper):
        x[:, :, 2 * off : 2 * off + sz] = p3[:, :, off : off + sz]
        x[:, :, 2 * off + sz : 2 * (off + sz)] = t3[:, :, off : off + sz]
    return x


def _run(preds, targets, m=M, f=F, n_cores=N_CORES, nb=8, pb=2, taper=True,
         out_split=True, out_wait=False, **run_kwargs):
    """Shard, run on hardware, return (partials [n_cores,128,T], results)."""
    key = (m, f, nb, pb, taper, out_split, out_wait)
    if key not in _cache:
        _cache[key] = _build(m, f, nb=nb, pb=pb, taper=taper,
                             out_split=out_split, out_wait=out_wait)
    nc = _cache[key]
    x = _pack(preds, targets, m, f, n_cores, taper=taper)
    in_maps = [{"x": x[c]} for c in range(n_cores)]
    r = run_bass_kernel_spmd(nc, in_maps, core_ids=list(range(n_cores)), **run_kwargs)
    partials = np.stack([r.results[c]["o"] for c in range(n_cores)])
    return partials, r


def kernel(preds, targets):
    import os

    # Force tracing off: the NTFF profile hook isn't importable in a bare
    # container and BASS_TRACE=1 in the environment would crash the run.
    prev = os.environ.get("BASS_NEVER_TRACE")
    os.environ["BASS_NEVER_TRACE"] = "1"
    try:
        partials, _ = _run(preds, targets)
    finally:
        if prev is None:
            os.environ.pop("BASS_NEVER_TRACE", None)
        else:
            os.environ["BASS_NEVER_TRACE"] = prev
    n = preds.shape[0]
    loss = partials.astype(np.float64).sum() / np.float64(n + 1)
    return np.float32(loss)
